# revision 10
# baseline (speedup 1.0000x reference)
"""Trainium2 Bass kernel: complex batch normalization (nn_ComplexBatchNormalization).

Strategy (8 NeuronCores, data-parallel over batch):
  - Shard batch 32 -> 4 per core. Per-core pixel count NPIX = 4*56*56 = 12544.
  - Phase 1 (stats): stream pixel-major tiles [128 pix, F*256]; DVE computes
    xr*xr, xr*xi, xi*xi; gpsimd reduces the F pixel-groups (free axis); PE
    (ones-matmul) does the partition reduction of xr, xi directly and of the
    per-(p,c) product accumulators at the end.
  - AllReduce of the 5*C per-channel sums (5 KB) across 8 cores.
  - Phase 2 (scalar): per-channel 2x2 inverse sqrtm + gamma/beta folding into
    an affine map  yr = Arr*xr + Ari*xi + cr ;  yi = Air*xr + Aii*xi + ci.
    Coefficients broadcast to [128, C] tiles via K=1 PE matmul.
  - Phase 3 (apply): 8 elementwise ops per tile split DVE/gpsimd, final adds
    write interleaved (stride 2) into the output tile -> contiguous DMA out.
"""

import os
import sys

import numpy as np

for _p in ("/opt/trn_rl_repo", "/opt/pypackages"):
    if os.path.isdir(_p) and _p not in sys.path:
        sys.path.append(_p)

B, H, W, C = 32, 56, 56, 256
N_CORES = 8
B_PER = B // N_CORES
NPIX = B_PER * H * W  # per-core pixels
EPS = 1e-3

_cache = {}


def build_program(npix, f1, f2, n_cores, n_total_samples):
    """Build the per-core Bass program. npix must equal 128*f1*t1 = 128*f2*t2."""
    from contextlib import ExitStack

    import concourse.bacc as bacc
    import concourse.mybir as mybir
    import concourse.tile as tile
    from concourse.alu_op_type import AluOpType as alu

    dt = mybir.dt.float32
    act = mybir.ActivationFunctionType
    P = 128
    t1 = npix // (P * f1)
    t2 = npix // (P * f2)
    assert t1 * P * f1 == npix, (npix, f1)
    assert t2 * P * f2 == npix, (npix, f2)
    inv_n = 1.0 / float(n_total_samples)

    nc = bacc.Bacc(
        "TRN2", target_bir_lowering=False, debug=False, num_devices=n_cores
    )

    xr_d = nc.dram_tensor("x_real", [npix, C], dt, kind="ExternalInput")
    xi_d = nc.dram_tensor("x_imag", [npix, C], dt, kind="ExternalInput")
    gr_d = nc.dram_tensor("gamma_r", [1, C], dt, kind="ExternalInput")
    gi_d = nc.dram_tensor("gamma_i", [1, C], dt, kind="ExternalInput")
    br_d = nc.dram_tensor("beta_r", [1, C], dt, kind="ExternalInput")
    bi_d = nc.dram_tensor("beta_i", [1, C], dt, kind="ExternalInput")
    out_d = nc.dram_tensor("out", [npix, 2 * C], dt, kind="ExternalOutput")

    # tiled DRAM views
    xr_v1 = xr_d.ap().rearrange("(t p f) c -> t p (f c)", p=P, f=f1)
    xi_v1 = xi_d.ap().rearrange("(t p f) c -> t p (f c)", p=P, f=f1)
    xr_v2 = xr_d.ap().rearrange("(t p f) c -> t p (f c)", p=P, f=f2)
    xi_v2 = xi_d.ap().rearrange("(t p f) c -> t p (f c)", p=P, f=f2)
    out_v = out_d.ap().rearrange("(t p f) c -> t p (f c)", p=P, f=f2)

    FC1 = f1 * C
    FC2 = f2 * C

    with tile.TileContext(nc) as tc, ExitStack() as ctx:
        const_pool = ctx.enter_context(tc.tile_pool(name="const", bufs=1))
        acc_pool = ctx.enter_context(tc.tile_pool(name="acc", bufs=1))
        stat_pool = ctx.enter_context(tc.tile_pool(name="stat", bufs=1))
        bc_pool = ctx.enter_context(tc.tile_pool(name="bc", bufs=1))
        dram_pool = ctx.enter_context(tc.tile_pool(name="dram", bufs=1, space="DRAM"))

        ones_col = const_pool.tile([P, 1], dt)
        nc.vector.memset(ones_col[:], 1.0)
        ones_row = const_pool.tile([1, P], dt)
        nc.vector.memset(ones_row[:], 1.0)

        keys = ("rr", "ii", "tt")
        acc3 = acc_pool.tile([P, 3 * C], dt, tag="acc3", name="acc3")

        # ---------------- Phase 1: per-channel sums ----------------
        # squares via ACT (Square); cross term via identity
        #   2*sum(xr*xi) = sum((xr+xi)^2) - sum(xr^2) - sum(xi^2)
        with (
            tc.tile_pool(name="in1", bufs=3) as in1_pool,
            tc.tile_pool(name="prod", bufs=2) as prod_pool,
            tc.tile_pool(name="red", bufs=2) as red_pool,
            tc.tile_pool(name="ps1", bufs=1, space="PSUM") as ps1_pool,
        ):
            ps_sr = ps1_pool.tile([1, 512], dt, tag="ps_sr")
            ps_si = ps1_pool.tile([1, 512], dt, tag="ps_si")

            # slice boundaries (multiples of 256, max 512 wide)
            bounds = list(range(0, FC1, 512)) + [FC1]
            slices = [(bounds[j], min(bounds[j] + 512, FC1)) for j in range(len(bounds) - 1)]
            n_sl = len(slices)

            for i in range(t1):
                xr_t = in1_pool.tile([P, FC1], dt, tag="xr1")
                nc.sync.dma_start(xr_t[:], xr_v1[i])
                xi_t = in1_pool.tile([P, FC1], dt, tag="xi1")
                nc.sync.dma_start(xi_t[:], xi_v1[i])

                # PE: raw sums of xr and xi (partition reduction via ones)
                for j, (lo, hi) in enumerate(slices):
                    nc.tensor.matmul(
                        ps_sr[:, 0 : hi - lo],
                        ones_col[:],
                        xr_t[:, lo:hi],
                        start=(i == 0 and j == 0),
                        stop=(i == t1 - 1 and j == n_sl - 1),
                        skip_group_check=True,
                    )
                for j, (lo, hi) in enumerate(slices):
                    nc.tensor.matmul(
                        ps_si[:, 0 : hi - lo],
                        ones_col[:],
                        xi_t[:, lo:hi],
                        start=(i == 0 and j == 0),
                        stop=(i == t1 - 1 and j == n_sl - 1),
                        skip_group_check=True,
                    )

                # gpsimd: t = xr + xi ; ACT: three squares into one tile
                t_t = in1_pool.tile([P, FC1], dt, tag="t1s")
                nc.gpsimd.tensor_tensor(t_t[:], xr_t[:], xi_t[:], alu.add)
                sq_t = prod_pool.tile([P, 3 * FC1], dt, tag="sq")
                nc.scalar.activation(sq_t[:, 0:FC1], xr_t[:], act.Square)
                nc.scalar.activation(sq_t[:, FC1 : 2 * FC1], xi_t[:], act.Square)
                nc.scalar.activation(sq_t[:, 2 * FC1 : 3 * FC1], t_t[:], act.Square)

                # DVE: one fused f-fold over all three squares -> [P, 3C]
                r_t = red_pool.tile([P, 3 * C], dt, tag="r3")
                nc.vector.tensor_reduce(
                    r_t[:].rearrange("p (s c) -> p s c", s=3),
                    sq_t[:].rearrange("p (s f c) -> p s c f", s=3, f=f1),
                    axis=mybir.AxisListType.X,
                    op=alu.add,
                )
                if i == 0:
                    nc.gpsimd.tensor_copy(acc3[:], r_t[:])
                else:
                    nc.gpsimd.tensor_tensor(acc3[:], acc3[:], r_t[:], alu.add)

            # final partition reduction of the square accumulators
            ps_fin = {}
            for j, k in enumerate(keys):
                ps_fin[k] = ps1_pool.tile([1, C], dt, tag="psf_" + k, name="psf_" + k)
                nc.tensor.matmul(
                    ps_fin[k][:],
                    ones_col[:],
                    acc3[:, j * C : (j + 1) * C],
                    start=True,
                    stop=True,
                    skip_group_check=True,
                )

            # assemble [1, 5C] = [Sr | Si | Srr | Sii | Stt]
            stat_sb = stat_pool.tile([1, 5 * C], dt, tag="stat_loc")
            srf = stat_pool.tile([1, 512], dt, tag="fold_r")
            nc.scalar.copy(srf[:], ps_sr[:])
            nc.vector.tensor_tensor(
                stat_sb[:, 0:C], srf[:, 0:C], srf[:, C : 2 * C], alu.add
            )
            sif = stat_pool.tile([1, 512], dt, tag="fold_i")
            nc.scalar.copy(sif[:], ps_si[:])
            nc.vector.tensor_tensor(
                stat_sb[:, C : 2 * C], sif[:, 0:C], sif[:, C : 2 * C], alu.add
            )
            for j, k in enumerate(keys):
                nc.scalar.copy(stat_sb[:, (2 + j) * C : (3 + j) * C], ps_fin[k][:])

        # ---------------- AllReduce (5C floats) ----------------
        cc_in = dram_pool.tile([1, 5 * C], dt, tag="cc_in")
        cc_out = dram_pool.tile([1, 5 * C], dt, tag="cc_out")
        nc.sync.dma_start(cc_in[:], stat_sb[:])
        nc.gpsimd.collective_compute(
            "AllReduce",
            mybir.AluOpType.add,
            replica_groups=[list(range(n_cores))],
            ins=[cc_in.opt()],
            outs=[cc_out.opt()],
        )
        g_sb = stat_pool.tile([1, 5 * C], dt, tag="stat_glob")
        nc.sync.dma_start(g_sb[:], cc_out[:])

        # ---------------- Phase 2: per-channel coefficients ----------------
        sc_pool = ctx.enter_context(tc.tile_pool(name="sc", bufs=1))

        def new(tag):
            return sc_pool.tile([1, C], dt, tag=tag, name="sc_" + tag)

        def tt(out, a, b, op):
            nc.vector.tensor_tensor(out[:], a[:], b[:], op)

        def stt(out, in0, scalar, in1, op0, op1):
            nc.vector.scalar_tensor_tensor(out[:], in0[:], scalar, in1[:], op0, op1)

        mom = stat_pool.tile([1, 5 * C], dt, tag="mom")
        nc.vector.tensor_scalar_mul(mom[:], g_sb[:], inv_n)
        mr, mi = mom[:, 0:C], mom[:, C : 2 * C]
        err, eii, ett = (
            mom[:, 2 * C : 3 * C],
            mom[:, 3 * C : 4 * C],
            mom[:, 4 * C : 5 * C],
        )
        # eri = (ett - err - eii) / 2
        eri = new("eri")
        tt(eri, err, eii, alu.add)
        tt(eri, ett, eri, alu.subtract)
        nc.vector.tensor_scalar_mul(eri[:], eri[:], 0.5)

        vrr, vri, vii = new("vrr"), new("vri"), new("vii")
        tmp = new("tmp0")
        tt(tmp, mr, mr, alu.mult)
        nc.vector.scalar_tensor_tensor(vrr[:], tmp[:], -1.0, err, alu.mult, alu.add)
        nc.vector.tensor_scalar_add(vrr[:], vrr[:], EPS)
        tt(tmp, mr, mi, alu.mult)
        nc.vector.scalar_tensor_tensor(vri[:], tmp[:], -1.0, eri[:], alu.mult, alu.add)
        tt(tmp, mi, mi, alu.mult)
        nc.vector.scalar_tensor_tensor(vii[:], tmp[:], -1.0, eii, alu.mult, alu.add)
        nc.vector.tensor_scalar_add(vii[:], vii[:], EPS)

        det, tmp2 = new("det"), new("tmp2")
        tt(tmp, vrr, vii, alu.mult)
        tt(tmp2, vri, vri, alu.mult)
        stt(det, tmp2, -1.0, tmp, alu.mult, alu.add)  # det = vrr*vii - vri^2

        invdet, s_t = new("invdet"), new("s_t")
        nc.vector.reciprocal(invdet[:], det[:])
        nc.scalar.activation(s_t[:], invdet[:], act.Sqrt)  # s = 1/sqrt(det)

        a_t, d_t, pb = new("a_t"), new("d_t"), new("pb")
        tt(a_t, vii, invdet, alu.mult)
        tt(d_t, vrr, invdet, alu.mult)
        tt(pb, vri, invdet, alu.mult)  # pb = -b = vri/det

        tq, invt = new("tq"), new("invt")
        tt(tmp, a_t, d_t, alu.add)
        stt(tmp2, s_t, 2.0, tmp, alu.mult, alu.add)  # a + d + 2s
        nc.scalar.activation(tq[:], tmp2[:], act.Sqrt)
        nc.vector.reciprocal(invt[:], tq[:])

        s00, s11, s01m = new("s00"), new("s11"), new("s01m")
        tt(tmp, a_t, s_t, alu.add)
        tt(s00, tmp, invt, alu.mult)
        tt(tmp, d_t, s_t, alu.add)
        tt(s11, tmp, invt, alu.mult)
        tt(s01m, pb, invt, alu.mult)  # s01m = -s01

        gr_t, gi_t, br_t, bi_t = new("gr"), new("gi"), new("br"), new("bi")
        nc.sync.dma_start(gr_t[:], gr_d.ap())
        nc.sync.dma_start(gi_t[:], gi_d.ap())
        nc.sync.dma_start(br_t[:], br_d.ap())
        nc.sync.dma_start(bi_t[:], bi_d.ap())

        # Arr = gr*s00 + gi*s01m ; Ari = -(gr*s01m + gi*s11)
        # Air = gi*s00 - gr*s01m ; Aii = gr*s11 - gi*s01m
        coefs = {}
        t4, t5, t6 = new("t4"), new("t5"), new("t6")
        tt(t4, gr_t, s00, alu.mult)
        tt(t5, gi_t, s01m, alu.mult)
        coefs["Arr"] = new("Arr")
        tt(coefs["Arr"], t4, t5, alu.add)
        tt(t6, gr_t, s01m, alu.mult)
        tt(tmp, gi_t, s11, alu.mult)
        coefs["Ari"] = new("Ari")
        tt(tmp2, t6, tmp, alu.add)
        nc.vector.tensor_scalar_mul(coefs["Ari"][:], tmp2[:], -1.0)
        tt(tmp, gi_t, s00, alu.mult)
        coefs["Air"] = new("Air")
        tt(coefs["Air"], tmp, t6, alu.subtract)
        tt(tmp, gr_t, s11, alu.mult)
        coefs["Aii"] = new("Aii")
        tt(coefs["Aii"], tmp, t5, alu.subtract)

        # cr = br - Arr*mr - Ari*mi ; ci = bi - Air*mr - Aii*mi
        coefs["cr"] = new("cr")
        tt(tmp, coefs["Arr"], mr, alu.mult)
        tt(tmp2, coefs["Ari"], mi, alu.mult)
        tt(tmp, tmp, tmp2, alu.add)
        tt(coefs["cr"], br_t, tmp, alu.subtract)
        coefs["ci"] = new("ci")
        tt(tmp, coefs["Air"], mr, alu.mult)
        tt(tmp2, coefs["Aii"], mi, alu.mult)
        tt(tmp, tmp, tmp2, alu.add)
        tt(coefs["ci"], bi_t, tmp, alu.subtract)

        # broadcast to [P, C] via K=1 matmul
        bcs = {}
        with tc.tile_pool(name="psbc", bufs=2, space="PSUM") as psbc_pool:
            for k in ("Arr", "Ari", "Air", "Aii", "cr", "ci"):
                psb = psbc_pool.tile([P, C], dt, tag="psbc")
                nc.tensor.matmul(
                    psb[:], ones_row[:], coefs[k][:], start=True, stop=True,
                    skip_group_check=True,
                )
                bcs[k] = bc_pool.tile([P, C], dt, tag="bc_" + k, name="bc_" + k)
                nc.scalar.copy(bcs[k][:], psb[:])

        # ---------------- Phase 3: apply ----------------
        def b3(bc_tile):
            # [P, C] coefficient viewed as [P, f2, C] with 0-stride repeat
            return bc_tile[:].unsqueeze(1).broadcast_to([P, f2, C])

        with (
            tc.tile_pool(name="in2", bufs=3) as in2_pool,
            tc.tile_pool(name="sc2", bufs=2) as sc2_pool,
            tc.tile_pool(name="out2", bufs=3) as out2_pool,
        ):
            for i in range(t2):
                xr_t = in2_pool.tile([P, FC2], dt, tag="xr2")
                nc.sync.dma_start(xr_t[:], xr_v2[i])
                xi_t = in2_pool.tile([P, FC2], dt, tag="xi2")
                nc.sync.dma_start(xi_t[:], xi_v2[i])

                xr3 = xr_t[:].rearrange("p (f c) -> p f c", f=f2)
                xi3 = xi_t[:].rearrange("p (f c) -> p f c", f=f2)

                a1 = sc2_pool.tile([P, FC2], dt, tag="a1")
                a2 = sc2_pool.tile([P, FC2], dt, tag="a2")
                b1 = sc2_pool.tile([P, FC2], dt, tag="b1")
                b2 = sc2_pool.tile([P, FC2], dt, tag="b2")
                a1_3 = a1[:].rearrange("p (f c) -> p f c", f=f2)
                a2_3 = a2[:].rearrange("p (f c) -> p f c", f=f2)
                b1_3 = b1[:].rearrange("p (f c) -> p f c", f=f2)
                b2_3 = b2[:].rearrange("p (f c) -> p f c", f=f2)

                nc.vector.tensor_tensor(a1_3, xr3, b3(bcs["Arr"]), alu.mult)
                nc.vector.tensor_tensor(a2_3, xi3, b3(bcs["Ari"]), alu.mult)
                nc.gpsimd.tensor_tensor(b1_3, xr3, b3(bcs["Air"]), alu.mult)
                nc.vector.tensor_tensor(b2_3, xi3, b3(bcs["Aii"]), alu.mult)
                nc.vector.tensor_tensor(a1_3, a1_3, a2_3, alu.add)
                nc.gpsimd.tensor_tensor(b1_3, b1_3, b2_3, alu.add)

                o_t = out2_pool.tile([P, 2 * FC2], dt, tag="o")
                o4 = o_t[:].rearrange("p (f c e) -> p f c e", f=f2, c=C)
                nc.vector.tensor_tensor(o4[:, :, :, 0], a1_3, b3(bcs["cr"]), alu.add)
                nc.gpsimd.tensor_tensor(o4[:, :, :, 1], b1_3, b3(bcs["ci"]), alu.add)

                nc.sync.dma_start(out_v[i], o_t[:])

    nc.compile()
    return nc


def _get_program():
    key = "prog"
    if key not in _cache:
        _cache[key] = build_program(NPIX, 7, 7, N_CORES, B * H * W)
    return _cache[key]


def _make_in_maps(np_inputs):
    x_real = np.ascontiguousarray(np.asarray(np_inputs["x_real"], dtype=np.float32))
    x_imag = np.ascontiguousarray(np.asarray(np_inputs["x_imag"], dtype=np.float32))
    g_r = np.ascontiguousarray(
        np.asarray(np_inputs["gamma_r"], dtype=np.float32)
    ).reshape(1, C)
    g_i = np.ascontiguousarray(
        np.asarray(np_inputs["gamma_i"], dtype=np.float32)
    ).reshape(1, C)
    b_r = np.ascontiguousarray(
        np.asarray(np_inputs["beta_r"], dtype=np.float32)
    ).reshape(1, C)
    b_i = np.ascontiguousarray(
        np.asarray(np_inputs["beta_i"], dtype=np.float32)
    ).reshape(1, C)

    in_maps = []
    for c in range(N_CORES):
        sl = slice(c * B_PER, (c + 1) * B_PER)
        in_maps.append(
            {
                "x_real": np.ascontiguousarray(x_real[sl]).reshape(NPIX, C),
                "x_imag": np.ascontiguousarray(x_imag[sl]).reshape(NPIX, C),
                "gamma_r": g_r,
                "gamma_i": g_i,
                "beta_r": b_r,
                "beta_i": b_i,
            }
        )
    return in_maps


def kernel(x_real, x_imag, gamma_r, gamma_i, beta_r, beta_i):
    from concourse import bass_utils

    nc = _get_program()
    in_maps = _make_in_maps(
        {
            "x_real": x_real,
            "x_imag": x_imag,
            "gamma_r": gamma_r,
            "gamma_i": gamma_i,
            "beta_r": beta_r,
            "beta_i": beta_i,
        }
    )
    res = bass_utils.run_bass_kernel_spmd(nc, in_maps, list(range(N_CORES)))
    out = np.concatenate(
        [res.results[c]["out"].reshape(B_PER, H, W, C, 2) for c in range(N_CORES)],
        axis=0,
    )
    return out
